# revision 23
# baseline (speedup 1.0000x reference)
"""V8: V7 + MM1 padded to 128-row contraction. The HAM clock gate never
un-throttles for 4-row matmuls (measured: 427ns/MM forever), and one cold
MM1 per strip kept the whole PE at 1.2GHz. With W1 zero-padded to
[128,128] and an xT ring whose rows 4-127 are zeroed once, every matmul
is >=64-row and the PE sustains 2.4GHz (34ns small-MM pace measured)."""

import sys
import numpy as np

sys.path.insert(0, "/opt/trn_rl_repo")

N_CORES = 8
B_TOTAL, M, C, H = 65536, 16, 3, 128
B_CORE = B_TOTAL // N_CORES            # 8192
COLS = B_CORE * M                      # 131072
NB = 512
NQUAD = 8                              # 64-col quads per strip
NSTRIP = COLS // NB                    # 256
SUPER = 32                             # strips per output DMA
OUT_W = 16 * SUPER                     # 256 f32 cols per staging tile
CHUNK = 8                              # strips per input DMA
NRING = 8                              # hT ring depth
QA = 8                                 # relu quads on ACT (rest on DVE)

SCL2 = 128.0 * np.log2(np.e)
SC = float(np.sqrt(SCL2))
MAGIC = 16256.0 - 8.0 + 0.5
MASKOFF = 16000.0

_CACHE = {}


def _build(nstrip):
    import concourse.bacc as bacc
    import concourse.tile as tile
    from concourse import mybir

    f32 = mybir.dt.float32
    i16 = mybir.dt.int16
    DT = mybir.dt.bfloat16
    Alu = mybir.AluOpType
    Act = mybir.ActivationFunctionType

    nsuper = max(1, nstrip // SUPER)
    nchunk = (nstrip + CHUNK - 1) // CHUNK

    nc = bacc.Bacc("TRN2")
    probT = nc.dram_tensor("probT", [4, COLS], DT, kind="ExternalInput")
    w1 = nc.dram_tensor("w1", [128, H], DT, kind="ExternalInput")
    b1 = nc.dram_tensor("b1", [H, 1], f32, kind="ExternalInput")
    w2z8 = nc.dram_tensor("w2z8", [H, 4 * NQUAD], DT, kind="ExternalInput")
    c24 = nc.dram_tensor("c24", [H, 4], f32, kind="ExternalInput")
    bmask = nc.dram_tensor("bmask", [128, 64], f32, kind="ExternalInput")
    outb = nc.dram_tensor("outb", [nsuper, 128, OUT_W], f32,
                          kind="ExternalOutput")

    with tile.TileContext(nc) as tc:
        from contextlib import ExitStack
        with ExitStack() as ctx:
            singles = ctx.enter_context(tc.tile_pool(name="singles", bufs=1))
            pe = ctx.enter_context(tc.tile_pool(name="pe", bufs=5))
            pg = ctx.enter_context(tc.tile_pool(name="pg", bufs=5))
            pout = ctx.enter_context(tc.tile_pool(name="pout", bufs=2))
            pH = ctx.enter_context(tc.tile_pool(name="pH", bufs=3, space="PSUM"))
            pS = ctx.enter_context(tc.tile_pool(name="pS", bufs=5, space="PSUM"))

            w1_t = singles.tile([128, H], DT)
            nc.sync.dma_start(out=w1_t, in_=w1[:, :])
            b1_t = singles.tile([H, 1], f32)
            nc.scalar.dma_start(out=b1_t, in_=b1[:, :])
            c24_t = singles.tile([H, 4], f32)
            nc.scalar.dma_start(out=c24_t, in_=c24[:, :])
            bmask_t = singles.tile([128, 64], f32)
            nc.gpsimd.dma_start(out=bmask_t, in_=bmask[:, :])

            # hT ring: strip tiles [128, 8*68]; quad q occupies cols
            # 68q..68q+64 (h), 68q+64..68q+68 = w2z (preloaded once via one
            # contiguous DMA + cheap DVE strided scatters).
            w2z_t = singles.tile([128, 4 * NQUAD], DT)
            nc.gpsimd.dma_start(out=w2z_t, in_=w2z8[:, :])
            hTs = []
            for r in range(NRING):
                t = singles.tile([128, NQUAD * 68], DT, tag=f"hT{r}")
                tv = t[:, :].rearrange("q (p c) -> q p c", p=NQUAD)
                nc.vector.tensor_copy(
                    tv[:, :, 64:68],
                    w2z_t[:, :].rearrange("q (p c) -> q p c", p=NQUAD))
                hTs.append(t)

            xTs = []
            for r in range(3):
                t = singles.tile([128, NB * CHUNK], DT, tag=f"xT{r}")
                nc.vector.memset(t[:, :], 0)
                xTs.append(t)

            outS_box = {}
            chunks = set()

            def st_dma(c):
                if c >= nchunk or c in chunks:
                    return
                xT = xTs[c % 3]
                nc.sync.dma_start(out=xT[0:4, :],
                                  in_=probT[:, NB * CHUNK * c:
                                            NB * CHUNK * (c + 1)])
                chunks.add(c)

            def st_mm1(s):
                xT = xTs[(s // CHUNK) % 3]
                off = (s % CHUNK) * NB
                psumH = pH.tile([128, NB], f32, tag="psumH")
                nc.tensor.matmul(psumH[:, :], w1_t[:, :],
                                 xT[:, off:off + NB], start=True, stop=True)
                return psumH

            def st_relu(s, psumH):
                hT = hTs[s % NRING]
                hv = hT[:, :].rearrange("q (p c) -> q p c", p=NQUAD)
                pv = psumH[:, :].rearrange("q (p c) -> q p c", p=NQUAD)
                # ACT: quads [0, QA); DVE: quads [QA, 8)
                nc.scalar.activation(hv[:, 0:QA, 0:64], pv[:, 0:QA, :],
                                     Act.Relu, bias=b1_t[:, 0:1], scale=1.0)
                if QA < NQUAD:
                    nc.vector.tensor_scalar(hv[:, QA:NQUAD, 0:64],
                                            pv[:, QA:NQUAD, :],
                                            scalar1=b1_t[:, 0:1],
                                            scalar2=0.0, op0=Alu.add,
                                            op1=Alu.max)
                return hT

            def st_gram(s, hT):
                psumS = pS.tile([128, 288], f32, tag="psumS")
                for q in range(NQUAD):
                    j, g = q % 2, q // 2
                    nc.tensor.matmul(psumS[64 * j:64 * j + 64,
                                           68 * g:68 * g + 68],
                                     hT[:, 68 * q:68 * q + 64],
                                     hT[:, 68 * q:68 * q + 68],
                                     start=True, stop=True)
                return psumS

            def st_exp(s, psumS):
                sv = psumS[:, 0:272].rearrange("q (g c) -> q g c", g=4)
                Ei = pe.tile([128, 256], i16, tag="Ei")
                ev = Ei[:, :].rearrange("q (g c) -> q g c", g=4)
                nc.vector.tensor_tensor(
                    ev, sv[:, :, 0:64],
                    bmask_t[:, None, :].broadcast_to([128, 4, 64]),
                    op=Alu.add)
                return Ei

            def st_gpp(s, psumS):
                sv = psumS[:, 0:272].rearrange("q (g c) -> q g c", g=4)
                gpp = pg.tile([128, 16], DT, tag="gpp")
                gv = gpp[:, :].rearrange("q (g c) -> q g c", g=4)
                nc.vector.tensor_tensor(
                    gv, sv[:, :, 64:68],
                    c24_t[:, None, :].broadcast_to([128, 4, 4]),
                    op=Alu.add)
                return gpp

            def st_num(s, Ei, gpp, psumS):
                Em = Ei[:, :].bitcast(mybir.dt.bfloat16)
                for q in range(NQUAD):
                    j, g = q % 2, q // 2
                    nc.tensor.matmul(psumS[64 * j:64 * j + 64,
                                           272 + 4 * g:276 + 4 * g],
                                     Em[64 * j:64 * j + 64,
                                        64 * g:64 * g + 64],
                                     gpp[64 * j:64 * j + 64,
                                         4 * g:4 * g + 4],
                                     start=True, stop=True)

            def st_ocopy(s, psumS):
                su, t = divmod(s, SUPER)
                if t == 0:
                    outS = pout.tile([128, OUT_W], f32, tag="outS")
                    outS_box["t"] = outS
                outS = outS_box["t"]
                nc.vector.tensor_copy(outS[:, 16 * t:16 * (t + 1)],
                                      psumS[:, 272:288])
                if t == SUPER - 1 or s == nstrip - 1:
                    nc.sync.dma_start(out=outb[su, :, :], in_=outS[:, :])

            # depth-3 software pipeline
            live = {}
            st_dma(0)
            st_dma(1)
            for i in range(-4, nstrip + 4):
                if (i + 4) >= 0 and (i + 4) % CHUNK == 0:
                    st_dma((i + 4) // CHUNK + 2)
                if 0 <= i + 4 < nstrip:
                    psumH = st_mm1(i + 4)
                    hT = st_relu(i + 4, psumH)
                    live[i + 4] = [hT]
                if 0 <= i - 4 < nstrip:
                    st = live.pop(i - 4)
                    st_num(i - 4, st[2], st[3], st[1])
                    st_ocopy(i - 4, st[1])
                if 0 <= i < nstrip:
                    st = live[i]
                    psumS = st_gram(i, st[0])
                    st.append(psumS)
                    st.append(st_exp(i, psumS))
                    st.append(st_gpp(i, psumS))

    nc.finalize()
    return nc


def _prep_core_inputs(prob_core, W1, b1, W2, b2):
    import ml_dtypes
    bf16 = ml_dtypes.bfloat16
    pT = np.ascontiguousarray(prob_core.reshape(-1, C).T)
    idx = np.tile(np.arange(M, dtype=np.float32), B_CORE)[None]
    probT_aug = np.ascontiguousarray(np.concatenate([pT, idx], axis=0))

    W1s = np.zeros((128, H), np.float32)
    W1s[0:4] = np.asarray(W1, np.float32) * SC
    b1s = np.asarray(b1, np.float32).reshape(H, 1) * SC
    w2s = np.asarray(W2, np.float32) / SC
    w2z = np.concatenate([w2s, np.zeros((H, 1), np.float32)], axis=1)
    w2z8 = np.tile(w2z, (1, NQUAD))
    c24 = np.concatenate([np.broadcast_to(
        np.asarray(b2, np.float32)[None, :], (H, C)),
        np.ones((H, 1), np.float32)], axis=1)

    # quad mask [128, 64]: partition q vs col c same 16-element iff
    # (q % 64) // 16 == c // 16
    q = np.arange(128)[:, None]
    c = np.arange(64)[None, :]
    mask = (((q % 64) // 16) == (c // 16)).astype(np.float32)
    bmask = (MAGIC - MASKOFF * (1.0 - mask)).astype(np.float32)

    return {
        "probT": probT_aug.astype(bf16),
        "w1": np.ascontiguousarray(W1s).astype(bf16),
        "b1": np.ascontiguousarray(b1s),
        "w2z8": np.ascontiguousarray(w2z8).astype(bf16),
        "c24": np.ascontiguousarray(c24),
        "bmask": np.ascontiguousarray(bmask),
    }


def _postprocess(outb_arr):
    # outb [nsuper, 128, 256]: q = 64j + 16hi + m ; col = 16t + 4g + cc
    # batch-elem (in core) = 32*(16su + t) + 4*(2g + j) + hi ; member = m
    nsuper = outb_arr.shape[0]
    r = outb_arr.reshape(nsuper, 2, 4, 16, SUPER, 4, 4)  # su,j,hi,m,t,g,cc
    r = r.transpose(0, 4, 5, 1, 2, 3, 6)                 # su,t,g,j,hi,m,cc
    r = r.reshape(-1, M, 4)
    return np.ascontiguousarray(r[..., 0:3] / r[..., 3:4])


def kernel(prob, W1, b1, W2, b2, _trace=False):
    from concourse.bass_utils import run_bass_kernel_spmd

    if "nc" not in _CACHE:
        _CACHE["nc"] = _build(NSTRIP)
    nc = _CACHE["nc"]

    prob = np.asarray(prob, np.float32)
    in_maps = []
    for ci in range(N_CORES):
        pc = prob[ci * B_CORE:(ci + 1) * B_CORE]
        in_maps.append(_prep_core_inputs(pc, W1, b1, W2, b2))
    res = run_bass_kernel_spmd(nc, in_maps, list(range(N_CORES)),
                               trace=_trace)
    _CACHE["last_result"] = res
    out = np.zeros((B_TOTAL, M, C), np.float32)
    for ci in range(N_CORES):
        o = _postprocess(res.results[ci]["outb"])
        out[ci * B_CORE:ci * B_CORE + o.shape[0]] = o
    return out


# revision 24
# speedup vs baseline: 1.0938x; 1.0938x over previous
"""V8: V7 + MM1 padded to 128-row contraction. The HAM clock gate never
un-throttles for 4-row matmuls (measured: 427ns/MM forever), and one cold
MM1 per strip kept the whole PE at 1.2GHz. With W1 zero-padded to
[128,128] and an xT ring whose rows 4-127 are zeroed once, every matmul
is >=64-row and the PE sustains 2.4GHz (34ns small-MM pace measured)."""

import sys
import numpy as np

sys.path.insert(0, "/opt/trn_rl_repo")

N_CORES = 8
B_TOTAL, M, C, H = 65536, 16, 3, 128
B_CORE = B_TOTAL // N_CORES            # 8192
COLS = B_CORE * M                      # 131072
NB = 512
NQUAD = 8                              # 64-col quads per strip
NSTRIP = COLS // NB                    # 256
SUPER = 32                             # strips per output DMA
OUT_W = 16 * SUPER                     # 256 f32 cols per staging tile
CHUNK = 8                              # strips per input DMA
NRING = 8                              # hT ring depth
QA = 8                                 # relu quads on ACT (rest on DVE)

SCL2 = 128.0 * np.log2(np.e)
SC = float(np.sqrt(SCL2))
MAGIC = 16256.0 - 8.0 + 0.5
MASKOFF = 16000.0

_CACHE = {}


def _build(nstrip):
    import concourse.bacc as bacc
    import concourse.tile as tile
    from concourse import mybir

    f32 = mybir.dt.float32
    i16 = mybir.dt.int16
    DT = mybir.dt.bfloat16
    Alu = mybir.AluOpType
    Act = mybir.ActivationFunctionType

    nsuper = max(1, nstrip // SUPER)
    nchunk = (nstrip + CHUNK - 1) // CHUNK

    nc = bacc.Bacc("TRN2")
    probT = nc.dram_tensor("probT", [4, COLS], DT, kind="ExternalInput")
    w1 = nc.dram_tensor("w1", [128, H], DT, kind="ExternalInput")
    b1 = nc.dram_tensor("b1", [H, 1], f32, kind="ExternalInput")
    w2z8 = nc.dram_tensor("w2z8", [H, 4 * NQUAD], DT, kind="ExternalInput")
    c24 = nc.dram_tensor("c24", [H, 4], f32, kind="ExternalInput")
    bmask = nc.dram_tensor("bmask", [128, 64], f32, kind="ExternalInput")
    outb = nc.dram_tensor("outb", [nsuper, 128, OUT_W], f32,
                          kind="ExternalOutput")

    with tile.TileContext(nc) as tc:
        from contextlib import ExitStack
        with ExitStack() as ctx:
            singles = ctx.enter_context(tc.tile_pool(name="singles", bufs=1))
            pe = ctx.enter_context(tc.tile_pool(name="pe", bufs=4))
            pg = ctx.enter_context(tc.tile_pool(name="pg", bufs=4))
            pout = ctx.enter_context(tc.tile_pool(name="pout", bufs=2))
            pH = ctx.enter_context(tc.tile_pool(name="pH", bufs=4, space="PSUM"))
            pS = ctx.enter_context(tc.tile_pool(name="pS", bufs=4, space="PSUM"))

            w1_t = singles.tile([128, H], DT)
            nc.sync.dma_start(out=w1_t, in_=w1[:, :])
            b1_t = singles.tile([H, 1], f32)
            nc.scalar.dma_start(out=b1_t, in_=b1[:, :])
            c24_t = singles.tile([H, 4], f32)
            nc.scalar.dma_start(out=c24_t, in_=c24[:, :])
            bmask_t = singles.tile([128, 64], f32)
            nc.gpsimd.dma_start(out=bmask_t, in_=bmask[:, :])

            # hT ring: strip tiles [128, 8*68]; quad q occupies cols
            # 68q..68q+64 (h), 68q+64..68q+68 = w2z (preloaded once via one
            # contiguous DMA + cheap DVE strided scatters).
            w2z_t = singles.tile([128, 4 * NQUAD], DT)
            nc.gpsimd.dma_start(out=w2z_t, in_=w2z8[:, :])
            hTs = []
            for r in range(NRING):
                t = singles.tile([128, NQUAD * 68], DT, tag=f"hT{r}")
                tv = t[:, :].rearrange("q (p c) -> q p c", p=NQUAD)
                nc.vector.tensor_copy(
                    tv[:, :, 64:68],
                    w2z_t[:, :].rearrange("q (p c) -> q p c", p=NQUAD))
                hTs.append(t)

            xTs = []
            for r in range(3):
                t = singles.tile([128, NB * CHUNK], DT, tag=f"xT{r}")
                nc.vector.memset(t[:, :], 0)
                xTs.append(t)

            outS_box = {}
            chunks = set()

            def st_dma(c):
                if c >= nchunk or c in chunks:
                    return
                xT = xTs[c % 3]
                nc.sync.dma_start(out=xT[0:4, :],
                                  in_=probT[:, NB * CHUNK * c:
                                            NB * CHUNK * (c + 1)])
                chunks.add(c)

            def st_mm1(s):
                xT = xTs[(s // CHUNK) % 3]
                off = (s % CHUNK) * NB
                psumH = pH.tile([128, NB], f32, tag="psumH")
                nc.tensor.matmul(psumH[:, :], w1_t[:, :],
                                 xT[:, off:off + NB], start=True, stop=True)
                return psumH

            def st_relu(s, psumH):
                hT = hTs[s % NRING]
                hv = hT[:, :].rearrange("q (p c) -> q p c", p=NQUAD)
                pv = psumH[:, :].rearrange("q (p c) -> q p c", p=NQUAD)
                # ACT: quads [0, QA); DVE: quads [QA, 8)
                nc.scalar.activation(hv[:, 0:QA, 0:64], pv[:, 0:QA, :],
                                     Act.Relu, bias=b1_t[:, 0:1], scale=1.0)
                if QA < NQUAD:
                    nc.vector.tensor_scalar(hv[:, QA:NQUAD, 0:64],
                                            pv[:, QA:NQUAD, :],
                                            scalar1=b1_t[:, 0:1],
                                            scalar2=0.0, op0=Alu.add,
                                            op1=Alu.max)
                return hT

            def st_gram(s, hT):
                psumS = pS.tile([128, 288], f32, tag="psumS")
                for q in range(NQUAD):
                    j, g = q % 2, q // 2
                    nc.tensor.matmul(psumS[64 * j:64 * j + 64,
                                           68 * g:68 * g + 68],
                                     hT[:, 68 * q:68 * q + 64],
                                     hT[:, 68 * q:68 * q + 68],
                                     start=True, stop=True)
                return psumS

            def st_exp(s, psumS):
                sv = psumS[:, 0:272].rearrange("q (g c) -> q g c", g=4)
                Ei = pe.tile([128, 256], i16, tag="Ei")
                ev = Ei[:, :].rearrange("q (g c) -> q g c", g=4)
                nc.vector.tensor_tensor(
                    ev, sv[:, :, 0:64],
                    bmask_t[:, None, :].broadcast_to([128, 4, 64]),
                    op=Alu.add)
                return Ei

            def st_gpp(s, psumS):
                sv = psumS[:, 0:272].rearrange("q (g c) -> q g c", g=4)
                gpp = pg.tile([128, 16], DT, tag="gpp")
                gv = gpp[:, :].rearrange("q (g c) -> q g c", g=4)
                nc.vector.tensor_tensor(
                    gv, sv[:, :, 64:68],
                    c24_t[:, None, :].broadcast_to([128, 4, 4]),
                    op=Alu.add)
                return gpp

            def st_num(s, Ei, gpp, psumS):
                Em = Ei[:, :].bitcast(mybir.dt.bfloat16)
                for q in range(NQUAD):
                    j, g = q % 2, q // 2
                    nc.tensor.matmul(psumS[64 * j:64 * j + 64,
                                           272 + 4 * g:276 + 4 * g],
                                     Em[64 * j:64 * j + 64,
                                        64 * g:64 * g + 64],
                                     gpp[64 * j:64 * j + 64,
                                         4 * g:4 * g + 4],
                                     start=True, stop=True)

            def st_ocopy(s, psumS):
                su, t = divmod(s, SUPER)
                if t == 0:
                    outS = pout.tile([128, OUT_W], f32, tag="outS")
                    outS_box["t"] = outS
                outS = outS_box["t"]
                nc.vector.tensor_copy(outS[:, 16 * t:16 * (t + 1)],
                                      psumS[:, 272:288])
                if t == SUPER - 1 or s == nstrip - 1:
                    nc.sync.dma_start(out=outb[su, :, :], in_=outS[:, :])

            # depth-3 software pipeline
            live = {}
            st_dma(0)
            st_dma(1)
            for i in range(-4, nstrip + 3):
                if (i + 4) >= 0 and (i + 4) % CHUNK == 0:
                    st_dma((i + 4) // CHUNK + 2)
                if 0 <= i + 4 < nstrip:
                    psumH = st_mm1(i + 4)
                    hT = st_relu(i + 4, psumH)
                    live[i + 4] = [hT]
                if 0 <= i - 3 < nstrip:
                    st = live.pop(i - 3)
                    st_num(i - 3, st[2], st[3], st[1])
                    st_ocopy(i - 3, st[1])
                if 0 <= i < nstrip:
                    st = live[i]
                    psumS = st_gram(i, st[0])
                    st.append(psumS)
                    st.append(st_exp(i, psumS))
                    st.append(st_gpp(i, psumS))

    nc.finalize()
    return nc


def _prep_core_inputs(prob_core, W1, b1, W2, b2):
    import ml_dtypes
    bf16 = ml_dtypes.bfloat16
    pT = np.ascontiguousarray(prob_core.reshape(-1, C).T)
    idx = np.tile(np.arange(M, dtype=np.float32), B_CORE)[None]
    probT_aug = np.ascontiguousarray(np.concatenate([pT, idx], axis=0))

    W1s = np.zeros((128, H), np.float32)
    W1s[0:4] = np.asarray(W1, np.float32) * SC
    b1s = np.asarray(b1, np.float32).reshape(H, 1) * SC
    w2s = np.asarray(W2, np.float32) / SC
    w2z = np.concatenate([w2s, np.zeros((H, 1), np.float32)], axis=1)
    w2z8 = np.tile(w2z, (1, NQUAD))
    c24 = np.concatenate([np.broadcast_to(
        np.asarray(b2, np.float32)[None, :], (H, C)),
        np.ones((H, 1), np.float32)], axis=1)

    # quad mask [128, 64]: partition q vs col c same 16-element iff
    # (q % 64) // 16 == c // 16
    q = np.arange(128)[:, None]
    c = np.arange(64)[None, :]
    mask = (((q % 64) // 16) == (c // 16)).astype(np.float32)
    bmask = (MAGIC - MASKOFF * (1.0 - mask)).astype(np.float32)

    return {
        "probT": probT_aug.astype(bf16),
        "w1": np.ascontiguousarray(W1s).astype(bf16),
        "b1": np.ascontiguousarray(b1s),
        "w2z8": np.ascontiguousarray(w2z8).astype(bf16),
        "c24": np.ascontiguousarray(c24),
        "bmask": np.ascontiguousarray(bmask),
    }


def _postprocess(outb_arr):
    # outb [nsuper, 128, 256]: q = 64j + 16hi + m ; col = 16t + 4g + cc
    # batch-elem (in core) = 32*(16su + t) + 4*(2g + j) + hi ; member = m
    nsuper = outb_arr.shape[0]
    r = outb_arr.reshape(nsuper, 2, 4, 16, SUPER, 4, 4)  # su,j,hi,m,t,g,cc
    r = r.transpose(0, 4, 5, 1, 2, 3, 6)                 # su,t,g,j,hi,m,cc
    r = r.reshape(-1, M, 4)
    return np.ascontiguousarray(r[..., 0:3] / r[..., 3:4])


def kernel(prob, W1, b1, W2, b2, _trace=False):
    from concourse.bass_utils import run_bass_kernel_spmd

    if "nc" not in _CACHE:
        _CACHE["nc"] = _build(NSTRIP)
    nc = _CACHE["nc"]

    prob = np.asarray(prob, np.float32)
    in_maps = []
    for ci in range(N_CORES):
        pc = prob[ci * B_CORE:(ci + 1) * B_CORE]
        in_maps.append(_prep_core_inputs(pc, W1, b1, W2, b2))
    res = run_bass_kernel_spmd(nc, in_maps, list(range(N_CORES)),
                               trace=_trace)
    _CACHE["last_result"] = res
    out = np.zeros((B_TOTAL, M, C), np.float32)
    for ci in range(N_CORES):
        o = _postprocess(res.results[ci]["outb"])
        out[ci * B_CORE:ci * B_CORE + o.shape[0]] = o
    return out


# revision 26
# speedup vs baseline: 1.0956x; 1.0017x over previous
"""V8: V7 + MM1 padded to 128-row contraction. The HAM clock gate never
un-throttles for 4-row matmuls (measured: 427ns/MM forever), and one cold
MM1 per strip kept the whole PE at 1.2GHz. With W1 zero-padded to
[128,128] and an xT ring whose rows 4-127 are zeroed once, every matmul
is >=64-row and the PE sustains 2.4GHz (34ns small-MM pace measured)."""

import sys
import numpy as np

sys.path.insert(0, "/opt/trn_rl_repo")

N_CORES = 8
B_TOTAL, M, C, H = 65536, 16, 3, 128
B_CORE = B_TOTAL // N_CORES            # 8192
COLS = B_CORE * M                      # 131072
NB = 512
NQUAD = 8                              # 64-col quads per strip
NSTRIP = COLS // NB                    # 256
SUPER = 32                             # strips per output DMA
OUT_W = 16 * SUPER                     # 256 f32 cols per staging tile
CHUNK = 8                              # strips per input DMA
NRING = 8                              # hT ring depth
QA = 8                                 # relu quads on ACT (rest on DVE)

SCL2 = 128.0 * np.log2(np.e)
SC = float(np.sqrt(SCL2))
MAGIC = 16256.0 - 8.0 + 0.5
MASKOFF = 16000.0

_CACHE = {}


def _build(nstrip):
    import concourse.bacc as bacc
    import concourse.tile as tile
    from concourse import mybir

    f32 = mybir.dt.float32
    i16 = mybir.dt.int16
    DT = mybir.dt.bfloat16
    Alu = mybir.AluOpType
    Act = mybir.ActivationFunctionType

    nsuper = max(1, nstrip // SUPER)
    nchunk = (nstrip + CHUNK - 1) // CHUNK

    nc = bacc.Bacc("TRN2")
    probT = nc.dram_tensor("probT", [4, COLS], DT, kind="ExternalInput")
    w1 = nc.dram_tensor("w1", [128, H], DT, kind="ExternalInput")
    b1 = nc.dram_tensor("b1", [H, 1], f32, kind="ExternalInput")
    w2z8 = nc.dram_tensor("w2z8", [H, 4 * NQUAD], DT, kind="ExternalInput")
    c24 = nc.dram_tensor("c24", [H, 4], f32, kind="ExternalInput")
    bmask = nc.dram_tensor("bmask", [128, 64], f32, kind="ExternalInput")
    outb = nc.dram_tensor("outb", [nsuper, 128, OUT_W], f32,
                          kind="ExternalOutput")

    with tile.TileContext(nc) as tc:
        from contextlib import ExitStack
        with ExitStack() as ctx:
            singles = ctx.enter_context(tc.tile_pool(name="singles", bufs=1))
            pe = ctx.enter_context(tc.tile_pool(name="pe", bufs=4))
            pg = ctx.enter_context(tc.tile_pool(name="pg", bufs=4))
            pout = ctx.enter_context(tc.tile_pool(name="pout", bufs=2))
            pH = ctx.enter_context(tc.tile_pool(name="pH", bufs=4, space="PSUM"))
            pS = ctx.enter_context(tc.tile_pool(name="pS", bufs=4, space="PSUM"))

            w1_t = singles.tile([128, H], DT)
            nc.sync.dma_start(out=w1_t, in_=w1[:, :])
            b1_t = singles.tile([H, 1], f32)
            nc.scalar.dma_start(out=b1_t, in_=b1[:, :])
            c24_t = singles.tile([H, 4], f32)
            nc.scalar.dma_start(out=c24_t, in_=c24[:, :])
            bmask_t = singles.tile([128, 64], f32)
            nc.gpsimd.dma_start(out=bmask_t, in_=bmask[:, :])

            # hT ring: strip tiles [128, 8*68]; quad q occupies cols
            # 68q..68q+64 (h), 68q+64..68q+68 = w2z (preloaded once via one
            # contiguous DMA + cheap DVE strided scatters).
            w2z_t = singles.tile([128, 4 * NQUAD], DT)
            nc.gpsimd.dma_start(out=w2z_t, in_=w2z8[:, :])
            hTs = []
            for r in range(NRING):
                t = singles.tile([128, NQUAD * 68], DT, tag=f"hT{r}")
                tv = t[:, :].rearrange("q (p c) -> q p c", p=NQUAD)
                nc.vector.tensor_copy(
                    tv[:, :, 64:68],
                    w2z_t[:, :].rearrange("q (p c) -> q p c", p=NQUAD))
                hTs.append(t)

            xTs = []
            for r in range(3):
                t = singles.tile([128, NB * CHUNK], DT, tag=f"xT{r}")
                nc.vector.memset(t[:, :], 0)
                xTs.append(t)

            outS_box = {}
            chunks = set()

            def st_dma(c):
                if c >= nchunk or c in chunks:
                    return
                xT = xTs[c % 3]
                nc.sync.dma_start(out=xT[0:4, :],
                                  in_=probT[:, NB * CHUNK * c:
                                            NB * CHUNK * (c + 1)])
                chunks.add(c)

            def st_mm1(s):
                xT = xTs[(s // CHUNK) % 3]
                off = (s % CHUNK) * NB
                psumH = pH.tile([128, NB], f32, tag="psumH")
                nc.tensor.matmul(psumH[:, :], w1_t[:, :],
                                 xT[:, off:off + NB], start=True, stop=True)
                return psumH

            def st_relu(s, psumH):
                hT = hTs[s % NRING]
                hv = hT[:, :].rearrange("q (p c) -> q p c", p=NQUAD)
                pv = psumH[:, :].rearrange("q (p c) -> q p c", p=NQUAD)
                # ACT: quads [0, QA); DVE: quads [QA, 8)
                nc.scalar.activation(hv[:, 0:QA, 0:64], pv[:, 0:QA, :],
                                     Act.Relu, bias=b1_t[:, 0:1], scale=1.0)
                if QA < NQUAD:
                    nc.vector.tensor_scalar(hv[:, QA:NQUAD, 0:64],
                                            pv[:, QA:NQUAD, :],
                                            scalar1=b1_t[:, 0:1],
                                            scalar2=0.0, op0=Alu.add,
                                            op1=Alu.max)
                return hT

            def st_gram(s, hT):
                psumS = pS.tile([128, 288], f32, tag="psumS")
                for q in range(NQUAD):
                    j, g = q % 2, q // 2
                    nc.tensor.matmul(psumS[64 * j:64 * j + 64,
                                           68 * g:68 * g + 68],
                                     hT[:, 68 * q:68 * q + 64],
                                     hT[:, 68 * q:68 * q + 68],
                                     start=True, stop=True)
                return psumS

            def st_exp(s, psumS):
                sv = psumS[:, 0:272].rearrange("q (g c) -> q g c", g=4)
                Ei = pe.tile([128, 256], i16, tag="Ei")
                ev = Ei[:, :].rearrange("q (g c) -> q g c", g=4)
                nc.vector.tensor_tensor(
                    ev, sv[:, :, 0:64],
                    bmask_t[:, None, :].broadcast_to([128, 4, 64]),
                    op=Alu.add)
                return Ei

            def st_gpp(s, psumS):
                sv = psumS[:, 0:272].rearrange("q (g c) -> q g c", g=4)
                gpp = pg.tile([128, 16], DT, tag="gpp")
                gv = gpp[:, :].rearrange("q (g c) -> q g c", g=4)
                nc.vector.tensor_tensor(
                    gv, sv[:, :, 64:68],
                    c24_t[:, None, :].broadcast_to([128, 4, 4]),
                    op=Alu.add)
                return gpp

            def st_num(s, Ei, gpp, psumS):
                Em = Ei[:, :].bitcast(mybir.dt.bfloat16)
                for q in range(NQUAD):
                    j, g = q % 2, q // 2
                    nc.tensor.matmul(psumS[64 * j:64 * j + 64,
                                           272 + 4 * g:276 + 4 * g],
                                     Em[64 * j:64 * j + 64,
                                        64 * g:64 * g + 64],
                                     gpp[64 * j:64 * j + 64,
                                         4 * g:4 * g + 4],
                                     start=True, stop=True)

            def st_ocopy(s, psumS):
                su, t = divmod(s, SUPER)
                if t == 0:
                    outS = pout.tile([128, OUT_W], f32, tag="outS")
                    outS_box["t"] = outS
                outS = outS_box["t"]
                nc.vector.tensor_copy(outS[:, 16 * t:16 * (t + 1)],
                                      psumS[:, 272:288])
                if t == SUPER - 1 or s == nstrip - 1:
                    nc.sync.dma_start(out=outb[su, :, :], in_=outS[:, :])

            # depth-3 software pipeline
            live = {}
            st_dma(0)
            st_dma(1)
            for i in range(-4, nstrip + 3):
                if (i + 4) >= 0 and (i + 4) % CHUNK == 0:
                    st_dma((i + 4) // CHUNK + 2)
                if 0 <= i + 4 < nstrip:
                    psumH = st_mm1(i + 4)
                    hT = st_relu(i + 4, psumH)
                    live[i + 4] = [hT]
                if 0 <= i - 3 < nstrip:
                    st = live.pop(i - 3)
                    st_num(i - 3, st[2], st[3], st[1])
                    st_ocopy(i - 3, st[1])
                if 0 <= i < nstrip:
                    st = live[i]
                    psumS = st_gram(i, st[0])
                    st.append(psumS)
                    st.append(st_exp(i, psumS))
                    st.append(st_gpp(i, psumS))

    nc.finalize()
    return nc


def _prep_core_inputs(prob_core, W1, b1, W2, b2):
    import ml_dtypes
    bf16 = ml_dtypes.bfloat16
    pT = np.ascontiguousarray(prob_core.reshape(-1, C).T)
    idx = np.tile(np.arange(M, dtype=np.float32), B_CORE)[None]
    probT_aug = np.ascontiguousarray(np.concatenate([pT, idx], axis=0))

    W1s = np.zeros((128, H), np.float32)
    W1s[0:4] = np.asarray(W1, np.float32) * SC
    b1s = np.asarray(b1, np.float32).reshape(H, 1) * SC
    w2s = np.asarray(W2, np.float32) / SC
    w2z = np.concatenate([w2s, np.zeros((H, 1), np.float32)], axis=1)
    w2z8 = np.tile(w2z, (1, NQUAD))
    c24 = np.concatenate([np.broadcast_to(
        np.asarray(b2, np.float32)[None, :], (H, C)),
        np.ones((H, 1), np.float32)], axis=1)

    # quad mask [128, 64]: partition q vs col c same 16-element iff
    # (q % 64) // 16 == c // 16
    q = np.arange(128)[:, None]
    c = np.arange(64)[None, :]
    mask = (((q % 64) // 16) == (c // 16)).astype(np.float32)
    bmask = (MAGIC - MASKOFF * (1.0 - mask)).astype(np.float32)

    return {
        "probT": probT_aug.astype(bf16),
        "w1": np.ascontiguousarray(W1s).astype(bf16),
        "b1": np.ascontiguousarray(b1s),
        "w2z8": np.ascontiguousarray(w2z8).astype(bf16),
        "c24": np.ascontiguousarray(c24),
        "bmask": np.ascontiguousarray(bmask),
    }


def _postprocess(outb_arr):
    # outb [nsuper, 128, 256]: q = 64j + 16hi + m ; col = 16t + 4g + cc
    # batch-elem (in core) = 32*(16su + t) + 4*(2g + j) + hi ; member = m
    nsuper = outb_arr.shape[0]
    r = outb_arr.reshape(nsuper, 2, 4, 16, SUPER, 4, 4)  # su,j,hi,m,t,g,cc
    r = r.transpose(0, 4, 5, 1, 2, 3, 6)                 # su,t,g,j,hi,m,cc
    r = r.reshape(-1, M, 4)
    return np.ascontiguousarray(r[..., 0:3] / r[..., 3:4])


def kernel(prob, W1, b1, W2, b2, _trace=False):
    from concourse.bass_utils import run_bass_kernel_spmd

    if "nc" not in _CACHE:
        _CACHE["nc"] = _build(NSTRIP)
    nc = _CACHE["nc"]

    prob = np.asarray(prob, np.float32)
    in_maps = []
    for ci in range(N_CORES):
        pc = prob[ci * B_CORE:(ci + 1) * B_CORE]
        in_maps.append(_prep_core_inputs(pc, W1, b1, W2, b2))
    res = run_bass_kernel_spmd(nc, in_maps, list(range(N_CORES)),
                               trace=_trace)
    _CACHE["last_result"] = res
    out = np.zeros((B_TOTAL, M, C), np.float32)
    for ci in range(N_CORES):
        o = _postprocess(res.results[ci]["outb"])
        out[ci * B_CORE:ci * B_CORE + o.shape[0]] = o
    return out
